# revision 4
# baseline (speedup 1.0000x reference)
"""Trainium2 Bass kernel for a transformer decoder layer (nn_DecL_55482387529838).

Reference shapes: B=2, S=2048, D=512, H=8, DFF=2048, depth=64.
Returns (out3, aw1, aw2) like the reference.

Sharding (8 cores): core c handles batch b=c//4 and head-pair hp=c%4
(heads 2*hp, 2*hp+1) for both attentions (writes its aw slices), plus a
row-slice of the FFN/out3 (rows [512*hp, 512*hp+512) of its batch).
Cross-core reduction of the O-projection partials uses ReduceScatter
(+AllGather for attn1) over the 4-core batch groups, which keeps the
program identical on every core (the scattered slice is rank-selected).

All matmuls run in float32r (TF32-like, ~1.5e-4 rel err, full PE rate).
Causality is exploited structurally: masked logit blocks are never
computed and the aw1 upper triangle relies on zero-initialized output
buffers.  Attention is computed in both layouts ([sq,sk] for the
normalized aw DRAM writes, [sk,sq] for the aw@v contraction) to avoid
any on-chip transpose of the 256MB attention-weight tensors.
"""

import numpy as np

import concourse.bass as bass
import concourse.mybir as mybir
import concourse.tile as tile
from concourse import bacc
from concourse.bass_utils import run_bass_kernel_spmd
from concourse.masks import make_identity, make_lower_triangular, make_upper_triangular

B, S, D, H, DFF = 2, 2048, 512, 8, 2048
DEPTH = 64
HLOC = 2                    # heads per core
P = 128                     # partitions
SC = S // P                 # 16 s-chunks of 128
SBK = S // 512              # 4 s-blocks of 512
DC = D // P                 # 4 d-chunks
FC = DFF // P               # 16 dff-chunks
OWN = S // 4                # 512 own rows
OC = OWN // P               # 4 own chunks
SCALE = 1.0 / float(np.sqrt(DEPTH))
EPS = 1e-6

F32 = mybir.dt.float32
F32R = mybir.dt.float32r
AF = mybir.ActivationFunctionType
ALU = mybir.AluOpType

GROUPS = [[0, 1, 2, 3], [4, 5, 6, 7]]


def build():
    nc = bacc.Bacc("TRN2", target_bir_lowering=False, debug=False)

    x_in = nc.dram_tensor("x_in", [S, D], F32, kind="ExternalInput")
    enc_in = nc.dram_tensor("enc_in", [S, D], F32, kind="ExternalInput")
    x_own_in = nc.dram_tensor("x_own_in", [OWN, D], F32, kind="ExternalInput")
    wnames = ["wq1h", "wk1h", "wv1h", "wq2h", "wk2h", "wv2h"]
    wts = {n: nc.dram_tensor(n, [D, HLOC * DEPTH], F32, kind="ExternalInput")
           for n in wnames}
    wts["wo1h"] = nc.dram_tensor("wo1h", [HLOC * DEPTH, D], F32, kind="ExternalInput")
    wts["wo2h"] = nc.dram_tensor("wo2h", [HLOC * DEPTH, D], F32, kind="ExternalInput")
    wf1_in = nc.dram_tensor("wf1_in", [D, DFF], F32, kind="ExternalInput")
    wf2_in = nc.dram_tensor("wf2_in", [DFF, D], F32, kind="ExternalInput")
    lns = {n: nc.dram_tensor(n, [1, D], F32, kind="ExternalInput")
           for n in ["g1", "b1", "g2", "b2", "g3", "b3"]}

    aw1_p = nc.dram_tensor("aw1_p", [HLOC, S, S], F32, kind="ExternalOutput")
    aw2_p = nc.dram_tensor("aw2_p", [HLOC, S, S], F32, kind="ExternalOutput")
    out3_p = nc.dram_tensor("out3_p", [OWN, D], F32, kind="ExternalOutput")

    with tile.TileContext(nc) as tc:
        import contextlib
        with contextlib.ExitStack() as ctx:
            pools = {}
            for name, kw in [
                ("const", dict(bufs=1)),
                ("wt", dict(bufs=1)),
                ("big", dict(bufs=1)),
                ("tblk", dict(bufs=2)),     # streamed transposed blocks
                ("xchunk", dict(bufs=2)),
                ("ochunk", dict(bufs=2)),
                ("awbuf", dict(bufs=2)),
                ("ltbuf", dict(bufs=3)),
                ("bcast", dict(bufs=2)),
                ("small", dict(bufs=8)),
                ("ln", dict(bufs=2)),
                ("ps_a", dict(bufs=2, space="PSUM")),    # transposes + projections
                ("ps_log", dict(bufs=2, space="PSUM")),
                ("ps_lt", dict(bufs=2, space="PSUM")),
                ("ps_av", dict(bufs=2, space="PSUM")),
                ("dram", dict(bufs=1, space="DRAM")),
                ("recd", dict(bufs=2, space="DRAM")),
            ]:
                pools[name] = ctx.enter_context(tc.tile_pool(name=name, **kw))

            # ---------------- constants ----------------
            ident = pools["const"].tile([P, P], F32)
            make_identity(nc, ident[:])
            tri_low = pools["const"].tile([P, P], F32)
            make_lower_triangular(nc, tri_low[:], val=1.0, diag=True)
            tri_up = pools["const"].tile([P, P], F32)
            make_upper_triangular(nc, tri_up[:], val=1.0, diag=True)
            eps_sb = pools["const"].tile([P, 1], F32)
            nc.vector.memset(eps_sb[:], EPS)
            ones16 = pools["const"].tile([P, SC, 1], F32)
            nc.vector.memset(ones16[:], 1.0)
            def _load_ln(gn, bn):
                g = pools["const"].tile([P, D], F32, tag="lng", name=f"ln_{gn}")
                nc.gpsimd.dma_start(out=g[:], in_=bass.AP(
                    tensor=lns[gn], offset=0, ap=[[0, P], [1, D]]))
                b = pools["const"].tile([P, D], F32, tag="lnb", name=f"ln_{bn}")
                nc.gpsimd.dma_start(out=b[:], in_=bass.AP(
                    tensor=lns[bn], offset=0, ap=[[0, P], [1, D]]))
                return g, b

            def _ln_chunk(t, g_sb, b_sb):
                stats = pools["small"].tile([P, 6], F32, tag="stats")
                nc.vector.bn_stats(out=stats[:], in_=t)
                mv = pools["small"].tile([P, 2], F32, tag="mv")
                nc.vector.bn_aggr(out=mv[:], in_=stats[:])
                nc.scalar.activation(out=mv[:, 1:2], in_=mv[:, 1:2], func=AF.Sqrt,
                                     bias=eps_sb[:], scale=1.0)
                nc.vector.reciprocal(out=mv[:, 1:2], in_=mv[:, 1:2])
                nc.vector.tensor_scalar(out=t, in0=t, scalar1=mv[:, 0:1],
                                        scalar2=mv[:, 1:2],
                                        op0=ALU.subtract, op1=ALU.mult)
                nc.vector.tensor_mul(out=t, in0=t, in1=g_sb[:])
                nc.vector.tensor_add(out=t, in0=t, in1=b_sb[:])

            def _transpose_block(chunks):
                """chunks: list of 4 [128, D] f32 APs (s-chunks of one 512-block).
                Returns tblk [P, DC, 512] f32r = transposed block."""
                tb = pools["tblk"].tile([P, DC, 512], F32R, tag="tblk")
                for j, ch in enumerate(chunks):
                    for dc in range(DC):
                        pt = pools["ps_a"].tile([P, P], F32, tag="pa")
                        nc.tensor.transpose(pt[:], ch[:, dc * P:(dc + 1) * P],
                                            ident[:])
                        nc.vector.tensor_copy(tb[:, dc, j * P:(j + 1) * P], pt[:])
                return tb

            def _proj_T_block(dst, w, tb, n):
                """dst[:, n*512:(n+1)*512] = w.T @ x for block n (dst [P,S] f32r)."""
                pp = pools["ps_a"].tile([P, 512], F32, tag="pa")
                for dc in range(DC):
                    nc.tensor.matmul(pp[:], w[:, dc, :], tb[:, dc, :],
                                     start=(dc == 0), stop=(dc == DC - 1))
                nc.vector.tensor_copy(dst[:, n * 512:(n + 1) * 512], pp[:])

            def _proj_v_block(dst, w, tb, n):
                """v natural chunks for block n into dst [P, SC, 2*65] f32r."""
                for j in range(4):
                    sc = n * 4 + j
                    pp = pools["ps_a"].tile([P, P], F32, tag="pa")
                    for dc in range(DC):
                        nc.tensor.matmul(pp[:], tb[:, dc, j * P:(j + 1) * P],
                                         w[:, dc, :],
                                         start=(dc == 0), stop=(dc == DC - 1))
                    for u in range(HLOC):
                        nc.vector.tensor_copy(
                            dst[:, sc, u * 65:u * 65 + DEPTH],
                            pp[:, u * DEPTH:(u + 1) * DEPTH])

            def _set_ones(v):
                for u in range(HLOC):
                    nc.vector.tensor_copy(v[:, :, u * 65 + DEPTH:u * 65 + DEPTH + 1],
                                          ones16[:])

            def _attention(qT, kT, v, attT, aw_out, causal):
                for u in range(HLOC):
                    qs = qT[u * DEPTH:(u + 1) * DEPTH, :]
                    ks = kT[u * DEPTH:(u + 1) * DEPTH, :]
                    # ---- write path: [sq, sk] ----
                    for qt in range(SC):
                        rowlen = (qt + 1) * P if causal else S
                        nfull = qt // 4 if causal else SBK
                        r = qt % 4
                        awb = pools["awbuf"].tile([P, S], F32, tag="awb")
                        asum = pools["small"].tile([P, SBK + 1], F32, tag="asum")
                        for kc in range(nfull):
                            lp = pools["ps_log"].tile([P, 512], F32, tag="lg")
                            nc.tensor.matmul(lp[:], qs[:, qt * P:(qt + 1) * P],
                                             ks[:, kc * 512:(kc + 1) * 512],
                                             start=True, stop=True)
                            nc.scalar.activation(
                                out=awb[:, kc * 512:(kc + 1) * 512], in_=lp[:],
                                func=AF.Exp, scale=SCALE,
                                accum_out=asum[:, kc:kc + 1])
                        if causal:
                            elen = (r + 1) * P
                            eoff = nfull * 512
                            lp = pools["ps_log"].tile([P, 512], F32, tag="lg")
                            nc.tensor.matmul(lp[:, 0:elen],
                                             qs[:, qt * P:(qt + 1) * P],
                                             ks[:, eoff:eoff + elen],
                                             start=True, stop=True)
                            nc.scalar.activation(out=awb[:, eoff:eoff + elen],
                                                 in_=lp[:, 0:elen],
                                                 func=AF.Exp, scale=SCALE)
                            doff = eoff + r * P
                            nc.vector.tensor_mul(out=awb[:, doff:doff + P],
                                                 in0=awb[:, doff:doff + P],
                                                 in1=tri_low[:])
                            nc.vector.tensor_reduce(
                                out=asum[:, nfull:nfull + 1],
                                in_=awb[:, eoff:eoff + elen],
                                axis=mybir.AxisListType.X, op=ALU.add)
                            nsum = nfull + 1
                        else:
                            nsum = nfull
                        tot = pools["small"].tile([P, 1], F32, tag="tot")
                        nc.vector.tensor_reduce(out=tot[:], in_=asum[:, 0:nsum],
                                                axis=mybir.AxisListType.X,
                                                op=ALU.add)
                        nc.vector.reciprocal(out=tot[:], in_=tot[:])
                        nc.vector.tensor_scalar_mul(out=awb[:, 0:rowlen],
                                                    in0=awb[:, 0:rowlen],
                                                    scalar1=tot[:])
                        nc.sync.dma_start(
                            out=aw_out[u, qt * P:(qt + 1) * P, 0:rowlen],
                            in_=awb[:, 0:rowlen])
                    # ---- av path: [sk, sq], ones-augmented v ----
                    for nb in range(SBK):
                        avp = pools["ps_av"].tile([DEPTH + 1, 512], F32, tag="av")
                        nkc = 4 * (nb + 1) if causal else SC
                        for kc in range(nkc):
                            co = max(0, kc * P - nb * 512) if causal else 0
                            n = 512 - co
                            ltp = pools["ps_lt"].tile([P, 512], F32, tag="lt")
                            nc.tensor.matmul(
                                ltp[:, 0:n], ks[:, kc * P:(kc + 1) * P],
                                qs[:, nb * 512 + co:(nb + 1) * 512],
                                start=True, stop=True)
                            elt = pools["ltbuf"].tile([P, 512], F32R, tag="elt")
                            nc.scalar.activation(out=elt[:, 0:n], in_=ltp[:, 0:n],
                                                 func=AF.Exp, scale=SCALE)
                            if causal and kc >= 4 * nb:
                                nc.vector.tensor_mul(out=elt[:, 0:P],
                                                     in0=elt[:, 0:P],
                                                     in1=tri_up[:])
                            nc.tensor.matmul(avp[:, co:512],
                                             v[:, kc, u * 65:u * 65 + 65],
                                             elt[:, 0:n], start=(kc == 0),
                                             stop=(kc == nkc - 1),
                                             skip_group_check=True)
                        rec = pools["small"].tile([1, 512], F32, tag="rec")
                        nc.vector.reciprocal(out=rec[:],
                                             in_=avp[DEPTH:DEPTH + 1, :])
                        rec_d = pools["recd"].tile([1, 512], F32, tag="recd")
                        nc.sync.dma_start(out=rec_d[:], in_=rec[:])
                        rd = rec_d[:]
                        bcast = pools["bcast"].tile([DEPTH, 512], F32, tag="bc")
                        nc.sync.dma_start(out=bcast[:], in_=bass.AP(
                            tensor=rd.tensor, offset=rd.offset,
                            ap=[[0, DEPTH], [1, 512]]))
                        nc.vector.tensor_mul(
                            out=attT[u * DEPTH:(u + 1) * DEPTH,
                                     nb * 512:(nb + 1) * 512],
                            in0=avp[0:DEPTH, :], in1=bcast[:])

            # ---------------- weights (mha1) ----------------
            w_sb = {}

            def _load_wqkv(name, tag):
                t = pools["wt"].tile([P, DC, HLOC * DEPTH], F32R, tag=tag)
                nc.gpsimd.dma_start(out=t[:], in_=wts[name][:].rearrange(
                    "(c p) m -> p c m", p=P))
                w_sb[name] = t

            def _load_wo(name, tag):
                t = pools["wt"].tile([P, D], F32R, tag=tag)
                nc.gpsimd.dma_start(out=t[:], in_=wts[name][:])
                w_sb[name] = t

            for n in ["wq1h", "wk1h", "wv1h"]:
                _load_wqkv(n, f"w_{n[1]}")
            _load_wo("wo1h", "w_o")

            # ---------------- phase 1+2: x -> xT blocks -> q1/k1/v1 ----------
            q1T = pools["big"].tile([P, S], F32R, tag="qT")
            k1T = pools["big"].tile([P, S], F32R, tag="kT")
            v1 = pools["big"].tile([P, SC, 2 * 65], F32R, tag="vv")
            for n in range(SBK):
                chunks = []
                for j in range(4):
                    sc = n * 4 + j
                    xc = pools["xchunk"].tile([P, D], F32, tag="xc")
                    nc.sync.dma_start(out=xc[:],
                                      in_=x_in[sc * P:(sc + 1) * P, :])
                    chunks.append(xc[:])
                tb = _transpose_block(chunks)
                _proj_T_block(q1T, w_sb["wq1h"], tb, n)
                _proj_T_block(k1T, w_sb["wk1h"], tb, n)
                _proj_v_block(v1, w_sb["wv1h"], tb, n)
            _set_ones(v1)

            # ---------------- phase 3: attn1 (causal) ----------------
            att1T = pools["big"].tile([P, S], F32R, tag="attT")
            _attention(q1T, k1T, v1, att1T, aw1_p, True)

            # ---------------- phase 4: O-proj1, RS+AG, LN1 ----------------
            o1_dram = pools["dram"].tile([S, D], F32, tag="o1d")
            for sc in range(SC):
                pp = pools["ps_a"].tile([P, 512], F32, tag="pa")
                nc.tensor.matmul(pp[:], att1T[:, sc * P:(sc + 1) * P],
                                 w_sb["wo1h"][:], start=True, stop=True)
                oc_sb = pools["ochunk"].tile([P, D], F32, tag="oc")
                nc.vector.tensor_copy(oc_sb[:], pp[:])
                nc.sync.dma_start(out=o1_dram[sc * P:(sc + 1) * P, :],
                                  in_=oc_sb[:])
            rs1_dram = pools["dram"].tile([OWN, D], F32, tag="rs1")
            nc.gpsimd.collective_compute(
                "ReduceScatter", ALU.add, replica_groups=GROUPS,
                ins=[o1_dram.opt()], outs=[rs1_dram.opt()])
            ag1_dram = pools["dram"].tile([S, D], F32, tag="ag1")
            nc.gpsimd.collective_compute(
                "AllGather", ALU.bypass, replica_groups=GROUPS,
                ins=[rs1_dram.opt()], outs=[ag1_dram.opt()])

            # own rows of out1
            g1_sb, b1_sb = _load_ln("g1", "b1")
            out1_own = pools["big"].tile([P, OC, D], F32, tag="o1own")
            for oc in range(OC):
                t2 = out1_own[:, oc, :]
                nc.sync.dma_start(out=t2, in_=rs1_dram[oc * P:(oc + 1) * P, :])
                xc = pools["xchunk"].tile([P, D], F32, tag="xc")
                nc.sync.dma_start(out=xc[:],
                                  in_=x_own_in[oc * P:(oc + 1) * P, :])
                nc.vector.tensor_add(out=t2, in0=t2, in1=xc[:])
                _ln_chunk(t2, g1_sb, b1_sb)

            # ---------------- phase 5: mha2 projections ----------------
            for n in ["wq2h", "wk2h", "wv2h"]:
                _load_wqkv(n, f"w_{n[1]}")
            _load_wo("wo2h", "w_o")
            q2T = pools["big"].tile([P, S], F32R, tag="qT")
            k2T = pools["big"].tile([P, S], F32R, tag="kT")
            v2 = pools["big"].tile([P, SC, 2 * 65], F32R, tag="vv")
            # out1 full -> LN1 -> transpose -> q2T (streamed per block)
            for n in range(SBK):
                chunks = []
                for j in range(4):
                    sc = n * 4 + j
                    t = pools["ln"].tile([P, D], F32, tag="lnc")
                    nc.sync.dma_start(out=t[:],
                                      in_=ag1_dram[sc * P:(sc + 1) * P, :])
                    xc = pools["xchunk"].tile([P, D], F32, tag="xc")
                    nc.sync.dma_start(out=xc[:],
                                      in_=x_in[sc * P:(sc + 1) * P, :])
                    nc.vector.tensor_add(out=t[:], in0=t[:], in1=xc[:])
                    _ln_chunk(t[:], g1_sb, b1_sb)
                    chunks.append(t[:])
                tb = _transpose_block(chunks)
                _proj_T_block(q2T, w_sb["wq2h"], tb, n)
            # enc -> encT blocks -> k2/v2
            for n in range(SBK):
                chunks = []
                for j in range(4):
                    sc = n * 4 + j
                    ec = pools["xchunk"].tile([P, D], F32, tag="xc")
                    nc.sync.dma_start(out=ec[:],
                                      in_=enc_in[sc * P:(sc + 1) * P, :])
                    chunks.append(ec[:])
                tb = _transpose_block(chunks)
                _proj_T_block(k2T, w_sb["wk2h"], tb, n)
                _proj_v_block(v2, w_sb["wv2h"], tb, n)
            _set_ones(v2)

            # ---------------- phase 6: attn2 (full) ----------------
            att2T = pools["big"].tile([P, S], F32R, tag="attT")
            _attention(q2T, k2T, v2, att2T, aw2_p, False)

            # ---------------- phase 7: O-proj2, RS2, LN2 (own) ----------------
            o2_dram = pools["dram"].tile([S, D], F32, tag="o2d")
            for sc in range(SC):
                pp = pools["ps_a"].tile([P, 512], F32, tag="pa")
                nc.tensor.matmul(pp[:], att2T[:, sc * P:(sc + 1) * P],
                                 w_sb["wo2h"][:], start=True, stop=True)
                oc_sb = pools["ochunk"].tile([P, D], F32, tag="oc")
                nc.vector.tensor_copy(oc_sb[:], pp[:])
                nc.sync.dma_start(out=o2_dram[sc * P:(sc + 1) * P, :],
                                  in_=oc_sb[:])
            rs2_dram = pools["dram"].tile([OWN, D], F32, tag="rs2")
            nc.gpsimd.collective_compute(
                "ReduceScatter", ALU.add, replica_groups=GROUPS,
                ins=[o2_dram.opt()], outs=[rs2_dram.opt()])

            g2_sb, b2_sb = _load_ln("g2", "b2")
            out2_own = pools["big"].tile([P, OC, D], F32, tag="o2own")
            out2T = pools["big"].tile([P, DC, OWN], F32R, tag="vv")
            for oc in range(OC):
                t2 = out2_own[:, oc, :]
                nc.sync.dma_start(out=t2, in_=rs2_dram[oc * P:(oc + 1) * P, :])
                nc.vector.tensor_add(out=t2, in0=t2, in1=out1_own[:, oc, :])
                _ln_chunk(t2, g2_sb, b2_sb)
                for dc in range(DC):
                    pt = pools["ps_a"].tile([P, P], F32, tag="pa")
                    nc.tensor.transpose(pt[:], t2[:, dc * P:(dc + 1) * P],
                                        ident[:])
                    nc.vector.tensor_copy(out2T[:, dc, oc * P:(oc + 1) * P],
                                          pt[:])

            # ---------------- phase 8: FFN (own rows) + LN3 ----------------
            wf1_sb = pools["big"].tile([P, DC, DFF], F32R, tag="qT")
            nc.gpsimd.dma_start(out=wf1_sb[:], in_=wf1_in[:].rearrange(
                "(c p) m -> p c m", p=P))
            wf2_sb = pools["big"].tile([P, FC, D], F32R, tag="kT")
            nc.gpsimd.dma_start(out=wf2_sb[:], in_=wf2_in[:].rearrange(
                "(c p) m -> p c m", p=P))
            g3_sb, b3_sb = _load_ln("g3", "b3")
            h1T = pools["big"].tile([P, FC, OWN], F32R, tag="attT")
            for fc in range(FC):
                pp = pools["ps_a"].tile([P, 512], F32, tag="pa")
                for dc in range(DC):
                    nc.tensor.matmul(pp[:], wf1_sb[:, dc, fc * P:(fc + 1) * P],
                                     out2T[:, dc, :],
                                     start=(dc == 0), stop=(dc == DC - 1))
                nc.scalar.activation(out=h1T[:, fc, :], in_=pp[:], func=AF.Relu)
            for oc in range(OC):
                pp = pools["ps_a"].tile([P, 512], F32, tag="pa")
                for fc in range(FC):
                    nc.tensor.matmul(pp[:], h1T[:, fc, oc * P:(oc + 1) * P],
                                     wf2_sb[:, fc, :],
                                     start=(fc == 0), stop=(fc == FC - 1))
                t = pools["ln"].tile([P, D], F32, tag="lnc")
                nc.vector.tensor_add(out=t[:], in0=pp[:],
                                     in1=out2_own[:, oc, :])
                _ln_chunk(t[:], g3_sb, b3_sb)
                nc.sync.dma_start(out=out3_p[oc * P:(oc + 1) * P, :], in_=t[:])

    nc.compile()
    return nc


_CACHED_NC = None


def kernel(**inputs):
    global _CACHED_NC
    if _CACHED_NC is None:
        _CACHED_NC = build()
    nc = _CACHED_NC

    f = lambda a: np.ascontiguousarray(np.asarray(a), dtype=np.float32)
    x = f(inputs["x"])
    enc = f(inputs["enc_output"])
    w = {n: f(inputs[n]) for n in ["wq1", "wk1", "wv1", "wo1",
                                   "wq2", "wk2", "wv2", "wo2",
                                   "wf1", "wf2"]}
    ln = {n: f(inputs[n]).reshape(1, D) for n in
          ["ln1_g", "ln1_b", "ln2_g", "ln2_b", "ln3_g", "ln3_b"]}

    in_maps = []
    for c in range(8):
        b, hp = c // 4, c % 4
        cs = slice(hp * HLOC * DEPTH, (hp + 1) * HLOC * DEPTH)
        in_maps.append({
            "x_in": x[b],
            "enc_in": enc[b],
            "x_own_in": x[b, hp * OWN:(hp + 1) * OWN],
            "wq1h": w["wq1"][:, cs], "wk1h": w["wk1"][:, cs],
            "wv1h": w["wv1"][:, cs],
            "wq2h": w["wq2"][:, cs], "wk2h": w["wk2"][:, cs],
            "wv2h": w["wv2"][:, cs],
            "wo1h": w["wo1"][cs, :], "wo2h": w["wo2"][cs, :],
            "wf1_in": w["wf1"], "wf2_in": w["wf2"],
            "g1": ln["ln1_g"], "b1": ln["ln1_b"],
            "g2": ln["ln2_g"], "b2": ln["ln2_b"],
            "g3": ln["ln3_g"], "b3": ln["ln3_b"],
        })

    res = run_bass_kernel_spmd(nc, in_maps, core_ids=list(range(8)))
    results = res.results

    out3 = np.zeros((B, S, D), np.float32)
    aw1 = np.zeros((B, H, S, S), np.float32)
    aw2 = np.zeros((B, H, S, S), np.float32)
    for c in range(8):
        b, hp = c // 4, c % 4
        aw1[b, 2 * hp:2 * hp + 2] = results[c]["aw1_p"]
        aw2[b, 2 * hp:2 * hp + 2] = results[c]["aw2_p"]
        out3[b, hp * OWN:(hp + 1) * OWN] = results[c]["out3_p"]
    return out3, aw1, aw2


# revision 9
# speedup vs baseline: 2.3912x; 2.3912x over previous
"""Trainium2 Bass kernel for a transformer decoder layer (nn_DecL_55482387529838).

Reference shapes: B=2, S=2048, D=512, H=8, DFF=2048, depth=64.
Returns (out3, aw1, aw2) like the reference.

Sharding (8 cores): core c handles batch b=c//4 and head-pair hp=c%4
(heads 2*hp, 2*hp+1) for both attentions (writes its aw slices), plus a
row-slice of the FFN/out3 (rows [512*hp, 512*hp+512) of its batch).
Cross-core reduction of the O-projection partials uses ReduceScatter
(+AllGather for attn1) over the 4-core batch groups, which keeps the
program identical on every core (the scattered slice is rank-selected).

All matmuls run in float32r (TF32-like, ~1.5e-4 rel err, full PE rate).
Causality is exploited structurally: masked logit blocks are never
computed and the aw1 upper triangle relies on zero-initialized output
buffers.  Attention is computed in both layouts ([sq,sk] for the
normalized aw DRAM writes, [sk,sq] for the aw@v contraction) to avoid
any on-chip transpose of the 256MB attention-weight tensors.
"""

import numpy as np

import concourse.bass as bass
import concourse.mybir as mybir
import concourse.tile as tile
from concourse import bacc
from concourse.bass_utils import run_bass_kernel_spmd
from concourse.masks import make_identity, make_lower_triangular, make_upper_triangular

B, S, D, H, DFF = 2, 2048, 512, 8, 2048
DEPTH = 64
HLOC = 2                    # heads per core
P = 128                     # partitions
SC = S // P                 # 16 s-chunks of 128
SBK = S // 512              # 4 s-blocks of 512
DC = D // P                 # 4 d-chunks
FC = DFF // P               # 16 dff-chunks
OWN = S // 4                # 512 own rows
OC = OWN // P               # 4 own chunks
SCALE = 1.0 / float(np.sqrt(DEPTH))
EPS = 1e-6

F32 = mybir.dt.float32
F32R = mybir.dt.float32r
AF = mybir.ActivationFunctionType
ALU = mybir.AluOpType

GROUPS = [[0, 1, 2, 3], [4, 5, 6, 7]]


def build(collectives=True):
    nc = bacc.Bacc("TRN2", target_bir_lowering=False, debug=False)

    x_in = nc.dram_tensor("x_in", [S, D], F32, kind="ExternalInput")
    x_own_in = nc.dram_tensor("x_own_in", [OWN, D], F32, kind="ExternalInput")
    x_t_in = nc.dram_tensor("x_t_in", [D, S], F32, kind="ExternalInput")
    enc_t_in = nc.dram_tensor("enc_t_in", [D, S], F32, kind="ExternalInput")
    wnames = ["wq1h", "wk1h", "wv1h", "wq2h", "wk2h", "wv2h"]
    wts = {n: nc.dram_tensor(n, [D, HLOC * DEPTH], F32, kind="ExternalInput")
           for n in wnames}
    wts["wo1h"] = nc.dram_tensor("wo1h", [HLOC * DEPTH, D], F32, kind="ExternalInput")
    wts["wo2h"] = nc.dram_tensor("wo2h", [HLOC * DEPTH, D], F32, kind="ExternalInput")
    wf1_in = nc.dram_tensor("wf1_in", [D, DFF], F32, kind="ExternalInput")
    wf2_in = nc.dram_tensor("wf2_in", [DFF, D], F32, kind="ExternalInput")
    lns = {n: nc.dram_tensor(n, [1, D], F32, kind="ExternalInput")
           for n in ["g1", "b1", "g2", "b2", "g3", "b3"]}

    aw1_p = nc.dram_tensor("aw1_p", [HLOC, S, S], F32, kind="ExternalOutput")
    aw2_p = nc.dram_tensor("aw2_p", [HLOC, S, S], F32, kind="ExternalOutput")
    out3_p = nc.dram_tensor("out3_p", [OWN, D], F32, kind="ExternalOutput")

    with tile.TileContext(nc) as tc:
        import contextlib
        with contextlib.ExitStack() as ctx:
            pools = {}
            for name, kw in [
                ("const", dict(bufs=1)),
                ("wt", dict(bufs=1)),
                ("big", dict(bufs=1)),
                ("tblk", dict(bufs=2)),     # streamed transposed blocks
                ("xchunk", dict(bufs=2)),
                ("ochunk", dict(bufs=2)),
                ("awbuf", dict(bufs=2)),
                ("ltbuf", dict(bufs=3)),
                ("bcast", dict(bufs=2)),
                ("small", dict(bufs=8)),
                ("ln", dict(bufs=2)),
                ("ps_a", dict(bufs=2, space="PSUM")),    # transposes + projections
                ("ps_log", dict(bufs=2, space="PSUM")),
                ("ps_lt", dict(bufs=2, space="PSUM")),
                ("ps_av", dict(bufs=2, space="PSUM")),
                ("dram", dict(bufs=1, space="DRAM")),
                ("recd", dict(bufs=2, space="DRAM")),
            ]:
                pools[name] = ctx.enter_context(tc.tile_pool(name=name, **kw))

            # ---------------- constants ----------------
            ident = pools["const"].tile([P, P], F32)
            make_identity(nc, ident[:])
            tri_low = pools["const"].tile([P, P], F32)
            make_lower_triangular(nc, tri_low[:], val=1.0, diag=True)
            tri_up = pools["const"].tile([P, P], F32)
            make_upper_triangular(nc, tri_up[:], val=1.0, diag=True)
            eps_sb = pools["const"].tile([P, 1], F32)
            nc.vector.memset(eps_sb[:], EPS)
            ones16 = pools["const"].tile([P, SC, 1], F32)
            nc.vector.memset(ones16[:], 1.0)
            def _load_ln(gn, bn):
                g = pools["const"].tile([P, D], F32, tag="lng", name=f"ln_{gn}")
                nc.gpsimd.dma_start(out=g[:], in_=bass.AP(
                    tensor=lns[gn], offset=0, ap=[[0, P], [1, D]]))
                b = pools["const"].tile([P, D], F32, tag="lnb", name=f"ln_{bn}")
                nc.gpsimd.dma_start(out=b[:], in_=bass.AP(
                    tensor=lns[bn], offset=0, ap=[[0, P], [1, D]]))
                return g, b

            def _ln_chunk(t, g_sb, b_sb):
                stats = pools["small"].tile([P, 6], F32, tag="stats")
                nc.vector.bn_stats(out=stats[:], in_=t)
                mv = pools["small"].tile([P, 2], F32, tag="mv")
                nc.vector.bn_aggr(out=mv[:], in_=stats[:])
                nc.scalar.activation(out=mv[:, 1:2], in_=mv[:, 1:2], func=AF.Sqrt,
                                     bias=eps_sb[:], scale=1.0)
                nc.vector.reciprocal(out=mv[:, 1:2], in_=mv[:, 1:2])
                nc.vector.tensor_scalar(out=t, in0=t, scalar1=mv[:, 0:1],
                                        scalar2=mv[:, 1:2],
                                        op0=ALU.subtract, op1=ALU.mult)
                nc.vector.tensor_mul(out=t, in0=t, in1=g_sb[:])
                nc.vector.tensor_add(out=t, in0=t, in1=b_sb[:])

            def _load_T_block(src_t, n):
                """tblk [P, DC, 512] f32r direct from pre-transposed DRAM."""
                tb = pools["tblk"].tile([P, DC, 512], F32R, tag="tblk")
                for dc in range(DC):
                    nc.gpsimd.dma_start(
                        out=tb[:, dc, :],
                        in_=src_t[dc * P:(dc + 1) * P, n * 512:(n + 1) * 512])
                return tb

            def _transpose_block(chunks):
                """chunks: list of 4 [128, D] f32 APs (s-chunks of one 512-block).
                Returns tblk [P, DC, 512] f32r = transposed block."""
                tb = pools["tblk"].tile([P, DC, 512], F32R, tag="tblk")
                for j, ch in enumerate(chunks):
                    for dc in range(DC):
                        pt = pools["ps_a"].tile([P, P], F32, tag="pa")
                        nc.tensor.transpose(pt[:], ch[:, dc * P:(dc + 1) * P],
                                            ident[:])
                        nc.vector.tensor_copy(tb[:, dc, j * P:(j + 1) * P], pt[:])
                return tb

            def _proj_T_block(dst, w, tb, n):
                """dst[:, n*512:(n+1)*512] = w.T @ x for block n (dst [P,S] f32r)."""
                pp = pools["ps_a"].tile([P, 512], F32, tag="pa")
                for dc in range(DC):
                    nc.tensor.matmul(pp[:], w[:, dc, :], tb[:, dc, :],
                                     start=(dc == 0), stop=(dc == DC - 1))
                nc.vector.tensor_copy(dst[:, n * 512:(n + 1) * 512], pp[:])

            def _proj_v_block(dst, w, tb, n):
                """v natural chunks for block n into dst [P, SC, 2*65] f32r."""
                for j in range(4):
                    sc = n * 4 + j
                    pp = pools["ps_a"].tile([P, P], F32, tag="pa")
                    for dc in range(DC):
                        nc.tensor.matmul(pp[:], tb[:, dc, j * P:(j + 1) * P],
                                         w[:, dc, :],
                                         start=(dc == 0), stop=(dc == DC - 1))
                    for u in range(HLOC):
                        nc.vector.tensor_copy(
                            dst[:, sc, u * 65:u * 65 + DEPTH],
                            pp[:, u * DEPTH:(u + 1) * DEPTH])

            def _set_ones(v):
                for u in range(HLOC):
                    nc.vector.tensor_copy(v[:, :, u * 65 + DEPTH:u * 65 + DEPTH + 1],
                                          ones16[:])

            def _attention(qT, kT, v, attT, aw_out, causal):
                for u in range(HLOC):
                    qs = qT[u * DEPTH:(u + 1) * DEPTH, :]
                    ks = kT[u * DEPTH:(u + 1) * DEPTH, :]
                    # ---- write path: [sq, sk] ----
                    for qt in range(SC):
                        rowlen = (qt + 1) * P if causal else S
                        nfull = qt // 4 if causal else SBK
                        r = qt % 4
                        awb = pools["awbuf"].tile([P, S], F32, tag="awb")
                        asum = pools["small"].tile([P, SBK + 1], F32, tag="asum")
                        for kc in range(nfull):
                            lp = pools["ps_log"].tile([P, 512], F32, tag="lg")
                            nc.tensor.matmul(lp[:], qs[:, qt * P:(qt + 1) * P],
                                             ks[:, kc * 512:(kc + 1) * 512],
                                             start=True, stop=True)
                            nc.scalar.activation(
                                out=awb[:, kc * 512:(kc + 1) * 512], in_=lp[:],
                                func=AF.Exp, scale=SCALE,
                                accum_out=asum[:, kc:kc + 1])
                        if causal:
                            elen = (r + 1) * P
                            eoff = nfull * 512
                            lp = pools["ps_log"].tile([P, 512], F32, tag="lg")
                            nc.tensor.matmul(lp[:, 0:elen],
                                             qs[:, qt * P:(qt + 1) * P],
                                             ks[:, eoff:eoff + elen],
                                             start=True, stop=True)
                            nc.scalar.activation(out=awb[:, eoff:eoff + elen],
                                                 in_=lp[:, 0:elen],
                                                 func=AF.Exp, scale=SCALE)
                            doff = eoff + r * P
                            nc.vector.tensor_mul(out=awb[:, doff:doff + P],
                                                 in0=awb[:, doff:doff + P],
                                                 in1=tri_low[:])
                            nc.vector.tensor_reduce(
                                out=asum[:, nfull:nfull + 1],
                                in_=awb[:, eoff:eoff + elen],
                                axis=mybir.AxisListType.X, op=ALU.add)
                            nsum = nfull + 1
                        else:
                            nsum = nfull
                        tot = pools["small"].tile([P, 1], F32, tag="tot")
                        nc.vector.tensor_reduce(out=tot[:], in_=asum[:, 0:nsum],
                                                axis=mybir.AxisListType.X,
                                                op=ALU.add)
                        nc.vector.reciprocal(out=tot[:], in_=tot[:])
                        nc.vector.tensor_scalar_mul(out=awb[:, 0:rowlen],
                                                    in0=awb[:, 0:rowlen],
                                                    scalar1=tot[:])
                        nc.sync.dma_start(
                            out=aw_out[u, qt * P:(qt + 1) * P, 0:rowlen],
                            in_=awb[:, 0:rowlen])
                    # ---- av path: [sk, sq], ones-augmented v ----
                    for nb in range(SBK):
                        avp = pools["ps_av"].tile([DEPTH + 1, 512], F32, tag="av")
                        nkc = 4 * (nb + 1) if causal else SC
                        for kc in range(nkc):
                            co = max(0, kc * P - nb * 512) if causal else 0
                            n = 512 - co
                            ltp = pools["ps_lt"].tile([P, 512], F32, tag="lt")
                            nc.tensor.matmul(
                                ltp[:, 0:n], ks[:, kc * P:(kc + 1) * P],
                                qs[:, nb * 512 + co:(nb + 1) * 512],
                                start=True, stop=True)
                            elt = pools["ltbuf"].tile([P, 512], F32R, tag="elt")
                            nc.scalar.activation(out=elt[:, 0:n], in_=ltp[:, 0:n],
                                                 func=AF.Exp, scale=SCALE)
                            if causal and kc >= 4 * nb:
                                nc.vector.tensor_mul(out=elt[:, 0:P],
                                                     in0=elt[:, 0:P],
                                                     in1=tri_up[:])
                            nc.tensor.matmul(avp[:, co:512],
                                             v[:, kc, u * 65:u * 65 + 65],
                                             elt[:, 0:n], start=(kc == 0),
                                             stop=(kc == nkc - 1),
                                             skip_group_check=True)
                        rec = pools["small"].tile([1, 512], F32, tag="rec")
                        nc.vector.reciprocal(out=rec[:],
                                             in_=avp[DEPTH:DEPTH + 1, :])
                        rec_d = pools["recd"].tile([1, 512], F32, tag="recd")
                        nc.sync.dma_start(out=rec_d[:], in_=rec[:])
                        rd = rec_d[:]
                        bcast = pools["bcast"].tile([DEPTH, 512], F32, tag="bc")
                        nc.sync.dma_start(out=bcast[:], in_=bass.AP(
                            tensor=rd.tensor, offset=rd.offset,
                            ap=[[0, DEPTH], [1, 512]]))
                        nc.vector.tensor_mul(
                            out=attT[u * DEPTH:(u + 1) * DEPTH,
                                     nb * 512:(nb + 1) * 512],
                            in0=avp[0:DEPTH, :], in1=bcast[:])

            # ---------------- weights (mha1) ----------------
            w_sb = {}

            def _load_wqkv(name, tag):
                t = pools["wt"].tile([P, DC, HLOC * DEPTH], F32R, tag=tag)
                nc.gpsimd.dma_start(out=t[:], in_=wts[name][:].rearrange(
                    "(c p) m -> p c m", p=P))
                w_sb[name] = t

            def _load_wo(name, tag):
                t = pools["wt"].tile([P, D], F32R, tag=tag)
                nc.gpsimd.dma_start(out=t[:], in_=wts[name][:])
                w_sb[name] = t

            for n in ["wq1h", "wk1h", "wv1h"]:
                _load_wqkv(n, f"w_{n[1]}")
            _load_wo("wo1h", "w_o")

            # ---------------- phase 1+2: x -> xT blocks -> q1/k1/v1 ----------
            q1T = pools["big"].tile([P, S], F32R, tag="qT")
            k1T = pools["big"].tile([P, S], F32R, tag="kT")
            v1 = pools["big"].tile([P, SC, 2 * 65], F32R, tag="vv")
            for n in range(SBK):
                tb = _load_T_block(x_t_in, n)
                _proj_T_block(q1T, w_sb["wq1h"], tb, n)
                _proj_T_block(k1T, w_sb["wk1h"], tb, n)
                _proj_v_block(v1, w_sb["wv1h"], tb, n)
            _set_ones(v1)

            # ---------------- phase 3: attn1 (causal) ----------------
            att1T = pools["big"].tile([P, S], F32R, tag="attT")
            _attention(q1T, k1T, v1, att1T, aw1_p, True)

            # ---------------- phase 4: O-proj1, RS+AG, LN1 ----------------
            o1_dram = pools["dram"].tile([S, D], F32, tag="o1d")
            for sc in range(SC):
                pp = pools["ps_a"].tile([P, 512], F32, tag="pa")
                nc.tensor.matmul(pp[:], att1T[:, sc * P:(sc + 1) * P],
                                 w_sb["wo1h"][:], start=True, stop=True)
                oc_sb = pools["ochunk"].tile([P, D], F32, tag="oc")
                nc.vector.tensor_copy(oc_sb[:], pp[:])
                nc.sync.dma_start(out=o1_dram[sc * P:(sc + 1) * P, :],
                                  in_=oc_sb[:])
            rs1_dram = pools["dram"].tile([OWN, D], F32, tag="rs1")
            ag1_dram = pools["dram"].tile([S, D], F32, tag="ag1")
            if collectives:
                nc.gpsimd.collective_compute(
                    "ReduceScatter", ALU.add, replica_groups=GROUPS,
                    ins=[o1_dram.opt()], outs=[rs1_dram.opt()])
                nc.gpsimd.collective_compute(
                    "AllGather", ALU.bypass, replica_groups=GROUPS,
                    ins=[rs1_dram.opt()], outs=[ag1_dram.opt()])
            else:
                nc.gpsimd.dma_start(out=rs1_dram[:], in_=o1_dram[0:OWN, :])
                nc.gpsimd.dma_start(out=ag1_dram[:], in_=o1_dram[:])

            # own rows of out1
            g1_sb, b1_sb = _load_ln("g1", "b1")
            out1_own = pools["big"].tile([P, OC, D], F32, tag="o1own")
            for oc in range(OC):
                t2 = out1_own[:, oc, :]
                nc.sync.dma_start(out=t2, in_=rs1_dram[oc * P:(oc + 1) * P, :])
                xc = pools["xchunk"].tile([P, D], F32, tag="xc")
                nc.sync.dma_start(out=xc[:],
                                  in_=x_own_in[oc * P:(oc + 1) * P, :])
                nc.vector.tensor_add(out=t2, in0=t2, in1=xc[:])
                _ln_chunk(t2, g1_sb, b1_sb)

            # ---------------- phase 5: mha2 projections ----------------
            for n in ["wq2h", "wk2h", "wv2h"]:
                _load_wqkv(n, f"w_{n[1]}")
            _load_wo("wo2h", "w_o")
            q2T = pools["big"].tile([P, S], F32R, tag="qT")
            k2T = pools["big"].tile([P, S], F32R, tag="kT")
            v2 = pools["big"].tile([P, SC, 2 * 65], F32R, tag="vv")
            # out1 full -> LN1 -> transpose -> q2T (streamed per block)
            for n in range(SBK):
                chunks = []
                for j in range(4):
                    sc = n * 4 + j
                    t = pools["ln"].tile([P, D], F32, tag="lnc")
                    nc.sync.dma_start(out=t[:],
                                      in_=ag1_dram[sc * P:(sc + 1) * P, :])
                    xc = pools["xchunk"].tile([P, D], F32, tag="xc")
                    nc.sync.dma_start(out=xc[:],
                                      in_=x_in[sc * P:(sc + 1) * P, :])
                    nc.vector.tensor_add(out=t[:], in0=t[:], in1=xc[:])
                    _ln_chunk(t[:], g1_sb, b1_sb)
                    chunks.append(t[:])
                tb = _transpose_block(chunks)
                _proj_T_block(q2T, w_sb["wq2h"], tb, n)
            # enc -> encT blocks -> k2/v2
            for n in range(SBK):
                tb = _load_T_block(enc_t_in, n)
                _proj_T_block(k2T, w_sb["wk2h"], tb, n)
                _proj_v_block(v2, w_sb["wv2h"], tb, n)
            _set_ones(v2)

            # ---------------- phase 6: attn2 (full) ----------------
            att2T = pools["big"].tile([P, S], F32R, tag="attT")
            _attention(q2T, k2T, v2, att2T, aw2_p, False)

            # ---------------- phase 7: O-proj2, RS2, LN2 (own) ----------------
            o2_dram = pools["dram"].tile([S, D], F32, tag="o2d")
            for sc in range(SC):
                pp = pools["ps_a"].tile([P, 512], F32, tag="pa")
                nc.tensor.matmul(pp[:], att2T[:, sc * P:(sc + 1) * P],
                                 w_sb["wo2h"][:], start=True, stop=True)
                oc_sb = pools["ochunk"].tile([P, D], F32, tag="oc")
                nc.vector.tensor_copy(oc_sb[:], pp[:])
                nc.sync.dma_start(out=o2_dram[sc * P:(sc + 1) * P, :],
                                  in_=oc_sb[:])
            rs2_dram = pools["dram"].tile([OWN, D], F32, tag="rs2")
            if collectives:
                nc.gpsimd.collective_compute(
                    "ReduceScatter", ALU.add, replica_groups=GROUPS,
                    ins=[o2_dram.opt()], outs=[rs2_dram.opt()])
            else:
                nc.gpsimd.dma_start(out=rs2_dram[:], in_=o2_dram[0:OWN, :])

            g2_sb, b2_sb = _load_ln("g2", "b2")
            out2_own = pools["big"].tile([P, OC, D], F32, tag="o2own")
            out2T = pools["big"].tile([P, DC, OWN], F32R, tag="vv")
            for oc in range(OC):
                t2 = out2_own[:, oc, :]
                nc.sync.dma_start(out=t2, in_=rs2_dram[oc * P:(oc + 1) * P, :])
                nc.vector.tensor_add(out=t2, in0=t2, in1=out1_own[:, oc, :])
                _ln_chunk(t2, g2_sb, b2_sb)
                for dc in range(DC):
                    pt = pools["ps_a"].tile([P, P], F32, tag="pa")
                    nc.tensor.transpose(pt[:], t2[:, dc * P:(dc + 1) * P],
                                        ident[:])
                    nc.vector.tensor_copy(out2T[:, dc, oc * P:(oc + 1) * P],
                                          pt[:])

            # ---------------- phase 8: FFN (own rows) + LN3 ----------------
            wf1_sb = pools["big"].tile([P, DC, DFF], F32R, tag="qT")
            nc.gpsimd.dma_start(out=wf1_sb[:], in_=wf1_in[:].rearrange(
                "(c p) m -> p c m", p=P))
            wf2_sb = pools["big"].tile([P, FC, D], F32R, tag="kT")
            nc.gpsimd.dma_start(out=wf2_sb[:], in_=wf2_in[:].rearrange(
                "(c p) m -> p c m", p=P))
            g3_sb, b3_sb = _load_ln("g3", "b3")
            h1T = pools["big"].tile([P, FC, OWN], F32R, tag="attT")
            for fc in range(FC):
                pp = pools["ps_a"].tile([P, 512], F32, tag="pa")
                for dc in range(DC):
                    nc.tensor.matmul(pp[:], wf1_sb[:, dc, fc * P:(fc + 1) * P],
                                     out2T[:, dc, :],
                                     start=(dc == 0), stop=(dc == DC - 1))
                nc.scalar.activation(out=h1T[:, fc, :], in_=pp[:], func=AF.Relu)
            for oc in range(OC):
                pp = pools["ps_a"].tile([P, 512], F32, tag="pa")
                for fc in range(FC):
                    nc.tensor.matmul(pp[:], h1T[:, fc, oc * P:(oc + 1) * P],
                                     wf2_sb[:, fc, :],
                                     start=(fc == 0), stop=(fc == FC - 1))
                t = pools["ln"].tile([P, D], F32, tag="lnc")
                nc.vector.tensor_add(out=t[:], in0=pp[:],
                                     in1=out2_own[:, oc, :])
                _ln_chunk(t[:], g3_sb, b3_sb)
                nc.sync.dma_start(out=out3_p[oc * P:(oc + 1) * P, :], in_=t[:])

    nc.compile()
    return nc


_CACHED_NC = None


def make_in_maps(inputs):
    f = lambda a: np.ascontiguousarray(np.asarray(a), dtype=np.float32)
    x = f(inputs["x"])
    enc = f(inputs["enc_output"])
    w = {n: f(inputs[n]) for n in ["wq1", "wk1", "wv1", "wo1",
                                   "wq2", "wk2", "wv2", "wo2",
                                   "wf1", "wf2"]}
    ln = {n: f(inputs[n]).reshape(1, D) for n in
          ["ln1_g", "ln1_b", "ln2_g", "ln2_b", "ln3_g", "ln3_b"]}

    in_maps = []
    for c in range(8):
        b, hp = c // 4, c % 4
        cs = slice(hp * HLOC * DEPTH, (hp + 1) * HLOC * DEPTH)
        in_maps.append({
            "x_in": x[b],
            "enc_in": enc[b],
            "x_own_in": x[b, hp * OWN:(hp + 1) * OWN],
            "x_t_in": np.ascontiguousarray(x[b].T),
            "enc_t_in": np.ascontiguousarray(enc[b].T),
            "wq1h": w["wq1"][:, cs], "wk1h": w["wk1"][:, cs],
            "wv1h": w["wv1"][:, cs],
            "wq2h": w["wq2"][:, cs], "wk2h": w["wk2"][:, cs],
            "wv2h": w["wv2"][:, cs],
            "wo1h": w["wo1"][cs, :], "wo2h": w["wo2"][cs, :],
            "wf1_in": w["wf1"], "wf2_in": w["wf2"],
            "g1": ln["ln1_g"], "b1": ln["ln1_b"],
            "g2": ln["ln2_g"], "b2": ln["ln2_b"],
            "g3": ln["ln3_g"], "b3": ln["ln3_b"],
        })
    return in_maps


def kernel(**inputs):
    global _CACHED_NC
    if _CACHED_NC is None:
        _CACHED_NC = build()
    nc = _CACHED_NC
    in_maps = make_in_maps(inputs)
    res = run_bass_kernel_spmd(nc, in_maps, core_ids=list(range(8)))
    results = res.results

    out3 = np.zeros((B, S, D), np.float32)
    aw1 = np.zeros((B, H, S, S), np.float32)
    aw2 = np.zeros((B, H, S, S), np.float32)
    for c in range(8):
        b, hp = c // 4, c % 4
        aw1[b, 2 * hp:2 * hp + 2] = results[c]["aw1_p"]
        aw2[b, 2 * hp:2 * hp + 2] = results[c]["aw2_p"]
        out3[b, hp * OWN:(hp + 1) * OWN] = results[c]["out3_p"]
    return out3, aw1, aw2
